# revision 32
# baseline (speedup 1.0000x reference)
"""BoT block (conv1x1+BN+ReLU -> 4-head MHSA+posemb -> conv1x1+BN -> residual+ReLU)
on 8 trn2 NeuronCores, data-parallel over batch (2 images per core).

v3: fp8(e4m3) DoubleRow matmuls for conv1/qk/v/att/conv3 (2x PE throughput);
attention in [d, x] layout (no transposes); softmax sums via a wide fp8 ones
matmul (sum pre-broadcast to all partitions) + fast reciprocal; image-
interleaved schedule so conv1(i+1)/conv3(i) matmuls fill the exp-bound
windows; scales chosen so feat/tails evict via 2-op vector tensor_scalar,
keeping the scalar engine exp-only in the critical stretches.

Self-contained: hardcodes shapes N=16, Cin=2048, H=W=32, heads=4, dqk=dv=128.
"""
import numpy as np
import ml_dtypes

import concourse.bass as bass
import concourse.mybir as mybir
import concourse.tile as tile
from concourse import bacc
from concourse.bass_utils import run_bass_kernel_spmd

EPS = 1e-5
HEADS = 4
DQK = 128
DV = 128
SCALE = DQK ** -0.5
N_IMG = 16
CIN = 2048
H = W = 32
HW = H * W            # 1024
MID = HEADS * DV      # 512
NCORES = 8
IMGS_PER_CORE = N_IMG // NCORES  # 2

P = 128
F8 = mybir.dt.float8e4
F16 = mybir.dt.float16
F32 = mybir.dt.float32
AF = mybir.ActivationFunctionType
ALU = mybir.AluOpType
DR = mybir.MatmulPerfMode.DoubleRow

KT1 = CIN // P        # 16 k-tiles for conv1
OT1 = MID // P        # 4 out-tiles for conv1
KT2 = MID // P        # 4 k-tiles for qk/v/conv3
OT3 = CIN // P        # 16 out-tiles for conv3
YT = HW // P          # 8 y-tiles
NH = HW // 512        # 2 halves of 512

SW1 = 32.0            # conv1 weight scale (feat = SW1*feat_true fits fp8)
SW2 = 1024.0          # qk/v weight scale (on top of 1/SW1 fold)
SW3 = 256.0           # conv3 weight scale
SA = 64.0             # fp8 attention-out scale (folded via reciprocal)
RES = SA * SW3        # identity-matmul residual scale (2^14)

_BUILT = {}
NP_F8 = ml_dtypes.float8_e4m3


def _build():
    if "nc" in _BUILT:
        return _BUILT["nc"]
    nc = bacc.Bacc("TRN2", target_bir_lowering=False, debug=False,
                   num_devices=NCORES)

    NCH = 4                      # k-tiles per DMA chunk
    NC1 = KT1 // NCH             # 4 x8/w1t chunks
    x8_d = nc.dram_tensor("x8", [IMGS_PER_CORE, NC1, P, NCH, HW], F8, kind="ExternalInput")
    xr_d = nc.dram_tensor("xr", [IMGS_PER_CORE, NC1, P, NCH, HW], F16, kind="ExternalInput")
    w1t_d = nc.dram_tensor("w1t", [NC1, P, NCH, MID], F8, kind="ExternalInput")
    qkwt_d = nc.dram_tensor("qkwt", [P, KT2, 2 * MID], F8, kind="ExternalInput")
    vwt_d = nc.dram_tensor("vwt", [P, KT2, MID], F8, kind="ExternalInput")
    w3t_d = nc.dram_tensor("w3t", [P, KT2, CIN], F8, kind="ExternalInput")
    embt_d = nc.dram_tensor("embt", [P, HW], F32, kind="ExternalInput")
    b1_d = nc.dram_tensor("b1", [P, OT1], F32, kind="ExternalInput")
    out_d = nc.dram_tensor("out", [IMGS_PER_CORE, OT3 // 2, P, 2, HW], F16,
                           kind="ExternalOutput")

    with tile.TileContext(nc) as tc:
        with (
            tc.tile_pool(name="consts", bufs=1) as consts,
            tc.tile_pool(name="xpool", bufs=8) as xpool,
            tc.tile_pool(name="feat", bufs=2) as featp,
            tc.tile_pool(name="qk", bufs=2) as qkp,
            tc.tile_pool(name="vaug", bufs=2) as vaugp,
            tc.tile_pool(name="et", bufs=5) as etp,
            tc.tile_pool(name="atf", bufs=2) as atfp,
            tc.tile_pool(name="rc", bufs=2) as rcp,
            tc.tile_pool(name="outp", bufs=2) as outp,
            tc.tile_pool(name="xres", bufs=4) as xresp,
            tc.tile_pool(name="ps_sml", bufs=4, space="PSUM") as ps_sml,
            tc.tile_pool(name="ps_big", bufs=2, space="PSUM") as ps_big,
        ):
            # ---- constants ----
            w1tc = [consts.tile([P, NCH, MID], F8, name=f"w1t_{j}")
                    for j in range(NC1)]
            b1 = consts.tile([P, OT1], F32)
            nc.sync.dma_start(b1[:], b1_d.ap())
            qkwt = consts.tile([P, KT2, 2 * MID], F8)
            vwt = consts.tile([P, KT2, MID], F8)
            w3t = consts.tile([P, KT2, CIN], F8)
            embt = consts.tile([P, HW], F32)
            ident = consts.tile([P, P], F16)
            ones2 = consts.tile([P, 2, P], F8)
            nc.gpsimd.memset(ident[:], 0.0)
            nc.gpsimd.affine_select(
                out=ident[:], in_=ident[:],
                compare_op=ALU.not_equal, fill=RES, base=0,
                pattern=[[-1, P]], channel_multiplier=1)
            nc.vector.memset(ones2[:], 1.0)

            state = {}

            def loadx(i, queues, with_w1t=False, fine=False):
                x8c = []
                for j in range(NC1):
                    t = xpool.tile([P, NCH, HW], F8, tag="x",
                                   name=f"x_{i}_{j}")
                    q = queues[j % len(queues)]
                    if fine and j == 0:
                        # halve the first chunk so conv1's chain starts early
                        q.dma_start(t[:, 0:2, :], x8_d.ap()[i, j][:, 0:2, :])
                        if with_w1t:
                            nc.gpsimd.dma_start(
                                w1tc[j][:, 0:2, :], w1t_d.ap()[j][:, 0:2, :])
                        q.dma_start(t[:, 2:4, :], x8_d.ap()[i, j][:, 2:4, :])
                        if with_w1t:
                            nc.gpsimd.dma_start(
                                w1tc[j][:, 2:4, :], w1t_d.ap()[j][:, 2:4, :])
                        x8c.append(t)
                        continue
                    q.dma_start(t[:], x8_d.ap()[i, j])
                    if with_w1t:
                        nc.gpsimd.dma_start(w1tc[j][:], w1t_d.ap()[j])
                    x8c.append(t)
                state[("x8", i)] = x8c

            def load_weights():
                nc.gpsimd.dma_start(qkwt[:], qkwt_d.ap())
                nc.gpsimd.dma_start(vwt[:], vwt_d.ap())
                nc.gpsimd.dma_start(w3t[:], w3t_d.ap())
                nc.gpsimd.dma_start(embt[:], embt_d.ap())

            def conv1(i, evict, fused=False):
                """chunk-major: concurrent psum chains consume each x8
                chunk as it lands; evict = f8(relu(psum + SW1*b1)).
                fused=True: both nh halves per chunk pass (8 chains, borrows
                the idle ps_big banks) -- only safe before logits claims
                them, i.e. for image 0."""
                x8c = state[("x8", i)]
                feat = featp.tile([P, KT2, HW], F8, tag="feat",
                                  name=f"feat_{i}")
                state[("feat", i)] = feat
                if fused:
                    with nc.named_scope(f"conv1_{i}"):
                        pss0 = [ps_sml.tile([P, 512], F32, tag="mm",
                                            name=f"c1f_{i}_{ot}")
                                for ot in range(OT1)]
                        psb = [ps_big.tile([P, 2, 512], F32, tag="lg",
                                           name=f"c1b_{i}_{j}")
                               for j in range(2)]
                        def chain(nh, ot):
                            return (pss0[ot][:] if nh == 0
                                    else psb[ot // 2][:, ot % 2, :])
                        for c in range(NC1):
                            for nh in range(NH):
                                sl = slice(nh * 512, (nh + 1) * 512)
                                for ot in range(OT1):
                                    for kk in range(0, NCH, 2):
                                        nc.tensor.matmul(
                                            chain(nh, ot),
                                            w1tc[c][:, kk:kk + 2,
                                                    ot * P:(ot + 1) * P],
                                            x8c[c][:, kk:kk + 2, sl],
                                            start=(c == 0 and kk == 0),
                                            stop=(c == NC1 - 1
                                                  and kk == NCH - 2),
                                            perf_mode=DR,
                                        )
                        for nh in range(NH):
                            sl = slice(nh * 512, (nh + 1) * 512)
                            for ot in range(OT1):
                                if evict == "scalar":
                                    nc.scalar.activation(
                                        feat[:, ot, sl], chain(nh, ot),
                                        AF.Relu, bias=b1[:, ot:ot + 1])
                                else:
                                    nc.vector.tensor_scalar(
                                        feat[:, ot, sl], chain(nh, ot),
                                        b1[:, ot:ot + 1], 0.0,
                                        ALU.add, ALU.max)
                    return
                with nc.named_scope(f"conv1_{i}"):
                    for nh in range(NH):
                        sl = slice(nh * 512, (nh + 1) * 512)
                        pss = [ps_sml.tile([P, 512], F32, tag="mm",
                                           name=f"c1_{i}_{nh}_{ot}")
                               for ot in range(OT1)]
                        for c in range(NC1):
                            for ot in range(OT1):
                                for kk in range(0, NCH, 2):
                                    nc.tensor.matmul(
                                        pss[ot][:],
                                        w1tc[c][:, kk:kk + 2,
                                                ot * P:(ot + 1) * P],
                                        x8c[c][:, kk:kk + 2, sl],
                                        start=(c == 0 and kk == 0),
                                        stop=(c == NC1 - 1 and kk == NCH - 2),
                                        perf_mode=DR,
                                    )
                        for ot in range(OT1):
                            if evict == "scalar":
                                nc.scalar.activation(
                                    feat[:, ot, sl], pss[ot][:], AF.Relu,
                                    bias=b1[:, ot:ot + 1])
                            else:
                                nc.vector.tensor_scalar(
                                    feat[:, ot, sl], pss[ot][:],
                                    b1[:, ot:ot + 1], 0.0, ALU.add, ALU.max)

            def qk(i):
                feat = state[("feat", i)]
                q_sb = qkp.tile([P, HEADS, HW], F16, tag="q", name=f"q_{i}")
                k_sb = qkp.tile([P, HEADS, HW], F16, tag="k", name=f"k_{i}")
                state[("q", i)], state[("k", i)] = q_sb, k_sb
                with nc.named_scope(f"qk_{i}"):
                    for ot in range(2 * HEADS):
                        for nh in range(NH):
                            sl = slice(nh * 512, (nh + 1) * 512)
                            ps = ps_sml.tile([P, 512], F32, tag="mm")
                            for kp in range(0, KT2, 2):
                                nc.tensor.matmul(
                                    ps[:],
                                    qkwt[:, kp:kp + 2, ot * P:(ot + 1) * P],
                                    feat[:, kp:kp + 2, sl],
                                    start=(kp == 0), stop=(kp == KT2 - 2),
                                    perf_mode=DR,
                                )
                            if ot < HEADS:
                                # scalar is exp-idle during qk stages
                                nc.scalar.activation(
                                    q_sb[:, ot, sl], ps[:], AF.Copy)
                            else:
                                nc.vector.tensor_tensor(
                                    k_sb[:, ot - HEADS, sl], ps[:],
                                    embt[:, sl], ALU.add)

            def logits_exp(i):
                q_sb, k_sb = state[("q", i)], state[("k", i)]
                ets = []
                for h in range(HEADS):
                    with nc.named_scope(f"logits_{i}_{h}"):
                        et = etp.tile([P, YT, HW], F8, tag="et",
                                      name=f"et_{i}_{h}")
                        ets.append(et)
                        for yj in range(YT):
                            lg = ps_big.tile([P, 2, 512], F32, tag="lg")
                            for xh in range(NH):
                                nc.tensor.matmul(
                                    lg[:, xh, :],
                                    k_sb[:, h, yj * P:(yj + 1) * P],
                                    q_sb[:, h, xh * 512:(xh + 1) * 512],
                                    start=True, stop=True,
                                )
                            nc.scalar.activation(
                                et[:, yj, :].rearrange(
                                    "p (a b) -> p a b", a=2),
                                lg[:, :, :], AF.Exp,
                                scale=1.0 / (SW2 * SW2))
                state[("et", i)] = ets

            def vstage(i):
                feat = state[("feat", i)]
                vaug = vaugp.tile([P, YT, HEADS, DV], F8, tag="vaug",
                                  name=f"vaug_{i}")
                state[("vaug", i)] = vaug
                with nc.named_scope(f"v_{i}"):
                    for yt in range(YT):
                        ps = ps_sml.tile([P, 512], F32, tag="mm")
                        for kp in range(0, KT2, 2):
                            nc.tensor.matmul(
                                ps[:],
                                feat[:, kp:kp + 2, yt * P:(yt + 1) * P],
                                vwt[:, kp:kp + 2, :],
                                start=(kp == 0), stop=(kp == KT2 - 2),
                                perf_mode=DR,
                            )
                        nc.vector.tensor_scalar(
                            vaug[:, yt, :, :],
                            ps[:].rearrange("p (h d) -> p h d", d=DV),
                            SA / SW2, None, ALU.mult)

            def xres_issue(i, queues):
                tiles = []
                for j in range(NC1):
                    xr_sb = xresp.tile([P, NCH, HW], F16, tag="xr",
                                       name=f"xr_{i}_{j}")
                    queues[j % len(queues)].dma_start(
                        xr_sb[:], xr_d.ap()[i, j])
                    tiles.append(xr_sb)
                state[("xres", i)] = tiles

            def atsm(i):
                vaug = state[("vaug", i)]
                atf = atfp.tile([P, KT2, HW], F8, tag="atf", name=f"atf_{i}")
                state[("atf", i)] = atf
                for h in range(HEADS):
                    with nc.named_scope(f"attn_{i}_{h}"):
                        et = state[("et", i)][h]
                        rcb = rcp.tile([P, NH, 512], F32, tag="rcb",
                                       name=f"rcb_{i}_{h}")
                        for xh in range(NH):
                            sl = slice(xh * 512, (xh + 1) * 512)
                            at = ps_sml.tile([P, 512], F32, tag="mm",
                                             name="at")
                            sm = ps_sml.tile([P, 512], F32, tag="mm",
                                             name="sm")
                            for yp in range(0, YT, 2):
                                nc.tensor.matmul(
                                    at[:],
                                    vaug[:, yp:yp + 2, h, :],
                                    et[:, yp:yp + 2, sl],
                                    start=(yp == 0), stop=(yp == YT - 2),
                                    perf_mode=DR,
                                )
                                nc.tensor.matmul(
                                    sm[:],
                                    ones2[:, :, :],
                                    et[:, yp:yp + 2, sl],
                                    start=(yp == 0), stop=(yp == YT - 2),
                                    perf_mode=DR,
                                )
                            nc.vector.reciprocal_approx_fast(
                                rcb[:, xh, :], sm[:])
                            nc.vector.scalar_tensor_tensor(
                                atf[:, h, sl], at[:], 0.0, rcb[:, xh, :],
                                ALU.max, ALU.mult)

            def conv3(i, ots, tail):
                """tail='split'|'vector'"""
                atf = state[("atf", i)]
                xres_tiles = state[("xres", i)]
                with nc.named_scope(f"conv3_{i}_{ots[0]}"):
                    o_sb = None
                    for ot in ots:
                        if ot % 2 == 0:
                            o_sb = outp.tile([P, 2, HW], F16, tag="o")
                        for nh in range(NH):
                            sl = slice(nh * 512, (nh + 1) * 512)
                            ps = ps_sml.tile([P, 512], F32, tag="mm")
                            for kp in range(0, KT2, 2):
                                nc.tensor.matmul(
                                    ps[:],
                                    w3t[:, kp:kp + 2, ot * P:(ot + 1) * P],
                                    atf[:, kp:kp + 2, sl],
                                    start=(kp == 0), stop=False,
                                    perf_mode=DR,
                                )
                            nc.tensor.matmul(
                                ps[:], ident[:],
                                xres_tiles[ot // NCH][:, ot % NCH, sl],
                                start=False, stop=True)
                            if tail == "split" and nh == 0:
                                nc.scalar.activation(
                                    o_sb[:, ot % 2, sl], ps[:], AF.Relu,
                                    scale=1.0 / RES)
                            else:
                                nc.vector.tensor_scalar(
                                    o_sb[:, ot % 2, sl], ps[:], 1.0 / RES,
                                    0.0, ALU.mult, ALU.max)
                        if ot % 2 == 1 and i == 1 and ot >= 11:
                            for half in range(2):
                                [nc.gpsimd, nc.sync][(ot + half) % 2].dma_start(
                                    out_d.ap()[i, ot // 2][:, half, :],
                                    o_sb[:, half, :])
                        elif ot % 2 == 1:
                            [nc.gpsimd, nc.sync][(ot // 2) % 2].dma_start(
                                out_d.ap()[i, ot // 2], o_sb[:])

            # ---- schedule (image-interleaved) ----
            loadx(0, [nc.sync, nc.scalar], with_w1t=True, fine=True)
            load_weights()
            conv1(0, evict="vector", fused=True)
            qk(0)
            logits_exp(0)
            vstage(0)
            xres_issue(0, [nc.gpsimd, nc.scalar])
            loadx(1, [nc.sync, nc.gpsimd])
            conv1(1, evict="vector")     # fills exp(0) window
            atsm(0)
            conv3(0, range(0, 6), tail="split")
            qk(1)
            logits_exp(1)
            conv3(0, range(6, 16), tail="vector")  # fills exp(1) window
            vstage(1)
            xres_issue(1, [nc.gpsimd, nc.scalar])
            atsm(1)
            conv3(1, range(OT3), tail="split")

    nc.compile()
    _BUILT["nc"] = nc
    return nc


def _prep_maps(x, conv1_w, gamma1, beta1, mean1, var1, qk_w, v_w, pos_h, pos_w,
               conv3_w, gamma3, beta3, mean3, var3):
    f16 = np.float16
    f32 = np.float32
    inv1 = (gamma1 / np.sqrt(var1 + EPS)).astype(f32)
    add1 = (beta1 - mean1 * inv1).astype(f32)
    inv3 = (gamma3 / np.sqrt(var3 + EPS)).astype(f32)
    add3 = (beta3 - mean3 * inv3).astype(f32)

    def q8(a, s):
        return np.ascontiguousarray(a * s).astype(NP_F8)

    NCH = 4
    NC1 = KT1 // NCH
    # [NC1, P, NCH, MID]: 4 k-tiles interleaved per partition row
    w1t = q8(conv1_w.T.reshape(NC1, NCH, P, MID).transpose(0, 2, 1, 3), SW1)
    qk_mod = np.concatenate(
        [qk_w[:HEADS * DQK] * SCALE, qk_w[HEADS * DQK:]], 0) * (inv1 / SW1)[None, :]
    qkwt = q8(qk_mod.T.reshape(KT2, P, 2 * MID).transpose(1, 0, 2), SW2)
    vwt = q8((v_w * (inv1 / SW1)[None, :]).T.reshape(KT2, P, MID).transpose(1, 0, 2), SW2)
    w3t = q8((conv3_w * inv3[:, None]).T.reshape(KT2, P, CIN).transpose(1, 0, 2), SW3)
    embt = np.ascontiguousarray(
        (pos_h[:, None, :] + pos_w[None, :, :]).reshape(HW, DQK).T * SW2
    ).astype(f32)
    # feat = relu(psum + SW1*(add1/inv1)) on device
    b1 = np.ascontiguousarray(
        (SW1 * add1 / inv1).reshape(OT1, P).T).astype(f32)

    xs = x.reshape(N_IMG, KT1, P, HW)
    xsc = xs.reshape(N_IMG, NC1, NCH, P, HW).transpose(0, 1, 3, 2, 4)
    x8_all = xsc.astype(NP_F8)
    xr_all = (xsc + add3.reshape(NC1, NCH, P).transpose(0, 2, 1)[None, :, :, :, None]
              ).astype(f16)

    in_maps = []
    for c in range(NCORES):
        sl = slice(c * IMGS_PER_CORE, (c + 1) * IMGS_PER_CORE)
        in_maps.append({
            "x8": np.ascontiguousarray(x8_all[sl]),
            "xr": np.ascontiguousarray(xr_all[sl]),
            "w1t": w1t, "qkwt": qkwt, "vwt": vwt, "w3t": w3t,
            "embt": embt, "b1": b1,
        })
    return in_maps


def _postprocess(out):
    # out: [N, OT3//2, P, 2, HW]; channel c = (pair*2 + half)*P + p
    out = out.transpose(0, 1, 3, 2, 4)
    return out.reshape(N_IMG, CIN, H, W).astype(np.float32)


def kernel(**inputs):
    nc = _build()
    inputs = {k: np.asarray(v) for k, v in inputs.items()}
    in_maps = _prep_maps(**inputs)
    res = run_bass_kernel_spmd(nc, in_maps, core_ids=list(range(NCORES)))
    out = np.concatenate([r["out"] for r in res.results], 0)
    return _postprocess(out)


# revision 33
# speedup vs baseline: 1.0108x; 1.0108x over previous
"""BoT block (conv1x1+BN+ReLU -> 4-head MHSA+posemb -> conv1x1+BN -> residual+ReLU)
on 8 trn2 NeuronCores, data-parallel over batch (2 images per core).

v3: fp8(e4m3) DoubleRow matmuls for conv1/qk/v/att/conv3 (2x PE throughput);
attention in [d, x] layout (no transposes); softmax sums via a wide fp8 ones
matmul (sum pre-broadcast to all partitions) + fast reciprocal; image-
interleaved schedule so conv1(i+1)/conv3(i) matmuls fill the exp-bound
windows; scales chosen so feat/tails evict via 2-op vector tensor_scalar,
keeping the scalar engine exp-only in the critical stretches.

Self-contained: hardcodes shapes N=16, Cin=2048, H=W=32, heads=4, dqk=dv=128.
"""
import numpy as np
import ml_dtypes

import concourse.bass as bass
import concourse.mybir as mybir
import concourse.tile as tile
from concourse import bacc
from concourse.bass_utils import run_bass_kernel_spmd

EPS = 1e-5
HEADS = 4
DQK = 128
DV = 128
SCALE = DQK ** -0.5
N_IMG = 16
CIN = 2048
H = W = 32
HW = H * W            # 1024
MID = HEADS * DV      # 512
NCORES = 8
IMGS_PER_CORE = N_IMG // NCORES  # 2

P = 128
F8 = mybir.dt.float8e4
F16 = mybir.dt.float16
F32 = mybir.dt.float32
AF = mybir.ActivationFunctionType
ALU = mybir.AluOpType
DR = mybir.MatmulPerfMode.DoubleRow

KT1 = CIN // P        # 16 k-tiles for conv1
OT1 = MID // P        # 4 out-tiles for conv1
KT2 = MID // P        # 4 k-tiles for qk/v/conv3
OT3 = CIN // P        # 16 out-tiles for conv3
YT = HW // P          # 8 y-tiles
NH = HW // 512        # 2 halves of 512

SW1 = 32.0            # conv1 weight scale (feat = SW1*feat_true fits fp8)
SW2 = 1024.0          # qk/v weight scale (on top of 1/SW1 fold)
SW3 = 256.0           # conv3 weight scale
SA = 64.0             # fp8 attention-out scale (folded via reciprocal)
RES = SA * SW3        # identity-matmul residual scale (2^14)

_BUILT = {}
NP_F8 = ml_dtypes.float8_e4m3


def _build():
    if "nc" in _BUILT:
        return _BUILT["nc"]
    nc = bacc.Bacc("TRN2", target_bir_lowering=False, debug=False,
                   num_devices=NCORES)

    NCH = 4                      # k-tiles per DMA chunk
    NC1 = KT1 // NCH             # 4 x8/w1t chunks
    x8_d = nc.dram_tensor("x8", [IMGS_PER_CORE, NC1, P, NCH, HW], F8, kind="ExternalInput")
    xr_d = nc.dram_tensor("xr", [IMGS_PER_CORE, NC1, P, NCH, HW], F16, kind="ExternalInput")
    w1t_d = nc.dram_tensor("w1t", [NC1, P, NCH, MID], F8, kind="ExternalInput")
    qkwt_d = nc.dram_tensor("qkwt", [P, KT2, 2 * MID], F8, kind="ExternalInput")
    vwt_d = nc.dram_tensor("vwt", [P, KT2, MID], F8, kind="ExternalInput")
    w3t_d = nc.dram_tensor("w3t", [P, KT2, CIN], F8, kind="ExternalInput")
    embt_d = nc.dram_tensor("embt", [P, HW], F32, kind="ExternalInput")
    b1_d = nc.dram_tensor("b1", [P, OT1], F32, kind="ExternalInput")
    out_d = nc.dram_tensor("out", [IMGS_PER_CORE, OT3 // 2, P, 2, HW], F16,
                           kind="ExternalOutput")

    with tile.TileContext(nc) as tc:
        with (
            tc.tile_pool(name="consts", bufs=1) as consts,
            tc.tile_pool(name="xpool", bufs=8) as xpool,
            tc.tile_pool(name="feat", bufs=2) as featp,
            tc.tile_pool(name="qk", bufs=2) as qkp,
            tc.tile_pool(name="vaug", bufs=2) as vaugp,
            tc.tile_pool(name="et", bufs=5) as etp,
            tc.tile_pool(name="atf", bufs=2) as atfp,
            tc.tile_pool(name="rc", bufs=2) as rcp,
            tc.tile_pool(name="outp", bufs=2) as outp,
            tc.tile_pool(name="xres", bufs=4) as xresp,
            tc.tile_pool(name="ps_sml", bufs=4, space="PSUM") as ps_sml,
            tc.tile_pool(name="ps_big", bufs=2, space="PSUM") as ps_big,
        ):
            # ---- constants ----
            w1tc = [consts.tile([P, NCH, MID], F8, name=f"w1t_{j}")
                    for j in range(NC1)]
            b1 = consts.tile([P, OT1], F32)
            nc.sync.dma_start(b1[:], b1_d.ap())
            qkwt = consts.tile([P, KT2, 2 * MID], F8)
            vwt = consts.tile([P, KT2, MID], F8)
            w3t = consts.tile([P, KT2, CIN], F8)
            embt = consts.tile([P, HW], F32)
            ident = consts.tile([P, P], F16)
            ones2 = consts.tile([P, 2, P], F8)
            nc.gpsimd.memset(ident[:], 0.0)
            nc.gpsimd.affine_select(
                out=ident[:], in_=ident[:],
                compare_op=ALU.not_equal, fill=RES, base=0,
                pattern=[[-1, P]], channel_multiplier=1)
            nc.vector.memset(ones2[:], 1.0)

            state = {}

            def loadx(i, queues, with_w1t=False, fine=False):
                x8c = []
                for j in range(NC1):
                    t = xpool.tile([P, NCH, HW], F8, tag="x",
                                   name=f"x_{i}_{j}")
                    q = queues[j % len(queues)]
                    if fine and j == 0:
                        # halve the first chunk so conv1's chain starts early
                        q.dma_start(t[:, 0:2, :], x8_d.ap()[i, j][:, 0:2, :])
                        if with_w1t:
                            nc.gpsimd.dma_start(
                                w1tc[j][:, 0:2, :], w1t_d.ap()[j][:, 0:2, :])
                        q.dma_start(t[:, 2:4, :], x8_d.ap()[i, j][:, 2:4, :])
                        if with_w1t:
                            nc.gpsimd.dma_start(
                                w1tc[j][:, 2:4, :], w1t_d.ap()[j][:, 2:4, :])
                        x8c.append(t)
                        continue
                    q.dma_start(t[:], x8_d.ap()[i, j])
                    if with_w1t:
                        nc.gpsimd.dma_start(w1tc[j][:], w1t_d.ap()[j])
                    x8c.append(t)
                state[("x8", i)] = x8c

            def load_weights():
                nc.gpsimd.dma_start(qkwt[:], qkwt_d.ap())
                nc.gpsimd.dma_start(vwt[:], vwt_d.ap())
                nc.gpsimd.dma_start(w3t[:], w3t_d.ap())
                nc.gpsimd.dma_start(embt[:], embt_d.ap())

            def conv1(i, evict, fused=False):
                """chunk-major: concurrent psum chains consume each x8
                chunk as it lands; evict = f8(relu(psum + SW1*b1)).
                fused=True: both nh halves per chunk pass (8 chains, borrows
                the idle ps_big banks) -- only safe before logits claims
                them, i.e. for image 0."""
                x8c = state[("x8", i)]
                feat = featp.tile([P, KT2, HW], F8, tag="feat",
                                  name=f"feat_{i}")
                state[("feat", i)] = feat
                if fused:
                    with nc.named_scope(f"conv1_{i}"):
                        pss0 = [ps_sml.tile([P, 512], F32, tag="mm",
                                            name=f"c1f_{i}_{ot}")
                                for ot in range(OT1)]
                        psb = [ps_big.tile([P, 2, 512], F32, tag="lg",
                                           name=f"c1b_{i}_{j}")
                               for j in range(2)]
                        def chain(nh, ot):
                            return (pss0[ot][:] if nh == 0
                                    else psb[ot // 2][:, ot % 2, :])
                        for c in range(NC1):
                            for nh in range(NH):
                                sl = slice(nh * 512, (nh + 1) * 512)
                                for ot in range(OT1):
                                    for kk in range(0, NCH, 2):
                                        nc.tensor.matmul(
                                            chain(nh, ot),
                                            w1tc[c][:, kk:kk + 2,
                                                    ot * P:(ot + 1) * P],
                                            x8c[c][:, kk:kk + 2, sl],
                                            start=(c == 0 and kk == 0),
                                            stop=(c == NC1 - 1
                                                  and kk == NCH - 2),
                                            perf_mode=DR,
                                        )
                        for nh in range(NH):
                            sl = slice(nh * 512, (nh + 1) * 512)
                            for ot in range(OT1):
                                if evict == "scalar":
                                    nc.scalar.activation(
                                        feat[:, ot, sl], chain(nh, ot),
                                        AF.Relu, bias=b1[:, ot:ot + 1])
                                else:
                                    nc.vector.tensor_scalar(
                                        feat[:, ot, sl], chain(nh, ot),
                                        b1[:, ot:ot + 1], 0.0,
                                        ALU.add, ALU.max)
                    return
                with nc.named_scope(f"conv1_{i}"):
                    for nh in range(NH):
                        sl = slice(nh * 512, (nh + 1) * 512)
                        pss = [ps_sml.tile([P, 512], F32, tag="mm",
                                           name=f"c1_{i}_{nh}_{ot}")
                               for ot in range(OT1)]
                        for c in range(NC1):
                            for ot in range(OT1):
                                for kk in range(0, NCH, 2):
                                    nc.tensor.matmul(
                                        pss[ot][:],
                                        w1tc[c][:, kk:kk + 2,
                                                ot * P:(ot + 1) * P],
                                        x8c[c][:, kk:kk + 2, sl],
                                        start=(c == 0 and kk == 0),
                                        stop=(c == NC1 - 1 and kk == NCH - 2),
                                        perf_mode=DR,
                                    )
                        for ot in range(OT1):
                            if evict == "scalar":
                                nc.scalar.activation(
                                    feat[:, ot, sl], pss[ot][:], AF.Relu,
                                    bias=b1[:, ot:ot + 1])
                            else:
                                nc.vector.tensor_scalar(
                                    feat[:, ot, sl], pss[ot][:],
                                    b1[:, ot:ot + 1], 0.0, ALU.add, ALU.max)

            def qk(i):
                feat = state[("feat", i)]
                q_sb = qkp.tile([P, HEADS, HW], F16, tag="q", name=f"q_{i}")
                k_sb = qkp.tile([P, HEADS, HW], F16, tag="k", name=f"k_{i}")
                state[("q", i)], state[("k", i)] = q_sb, k_sb
                with nc.named_scope(f"qk_{i}"):
                    for ot in range(2 * HEADS):
                        for nh in range(NH):
                            sl = slice(nh * 512, (nh + 1) * 512)
                            ps = ps_sml.tile([P, 512], F32, tag="mm")
                            for kp in range(0, KT2, 2):
                                nc.tensor.matmul(
                                    ps[:],
                                    qkwt[:, kp:kp + 2, ot * P:(ot + 1) * P],
                                    feat[:, kp:kp + 2, sl],
                                    start=(kp == 0), stop=(kp == KT2 - 2),
                                    perf_mode=DR,
                                )
                            if ot < HEADS:
                                # scalar is exp-idle during qk stages
                                nc.scalar.activation(
                                    q_sb[:, ot, sl], ps[:], AF.Copy)
                            else:
                                nc.vector.tensor_tensor(
                                    k_sb[:, ot - HEADS, sl], ps[:],
                                    embt[:, sl], ALU.add)

            def logits_exp(i):
                q_sb, k_sb = state[("q", i)], state[("k", i)]
                ets = []
                for h in range(HEADS):
                    with nc.named_scope(f"logits_{i}_{h}"):
                        et = etp.tile([P, YT, HW], F8, tag="et",
                                      name=f"et_{i}_{h}")
                        ets.append(et)
                        for yj in range(YT):
                            lg = ps_big.tile([P, 2, 512], F32, tag="lg")
                            for xh in range(NH):
                                nc.tensor.matmul(
                                    lg[:, xh, :],
                                    k_sb[:, h, yj * P:(yj + 1) * P],
                                    q_sb[:, h, xh * 512:(xh + 1) * 512],
                                    start=True, stop=True,
                                )
                            nc.scalar.activation(
                                et[:, yj, :].rearrange(
                                    "p (a b) -> p a b", a=2),
                                lg[:, :, :], AF.Exp,
                                scale=1.0 / (SW2 * SW2))
                state[("et", i)] = ets

            def vstage(i):
                feat = state[("feat", i)]
                vaug = vaugp.tile([P, YT, HEADS, DV], F8, tag="vaug",
                                  name=f"vaug_{i}")
                state[("vaug", i)] = vaug
                with nc.named_scope(f"v_{i}"):
                    for yt in range(YT):
                        ps = ps_sml.tile([P, 512], F32, tag="mm")
                        for kp in range(0, KT2, 2):
                            nc.tensor.matmul(
                                ps[:],
                                feat[:, kp:kp + 2, yt * P:(yt + 1) * P],
                                vwt[:, kp:kp + 2, :],
                                start=(kp == 0), stop=(kp == KT2 - 2),
                                perf_mode=DR,
                            )
                        nc.vector.tensor_scalar(
                            vaug[:, yt, :, :],
                            ps[:].rearrange("p (h d) -> p h d", d=DV),
                            SA / SW2, None, ALU.mult)

            def xres_issue(i, queues):
                tiles = []
                for j in range(NC1):
                    xr_sb = xresp.tile([P, NCH, HW], F16, tag="xr",
                                       name=f"xr_{i}_{j}")
                    queues[j % len(queues)].dma_start(
                        xr_sb[:], xr_d.ap()[i, j])
                    tiles.append(xr_sb)
                state[("xres", i)] = tiles

            def atsm(i):
                vaug = state[("vaug", i)]
                atf = atfp.tile([P, KT2, HW], F8, tag="atf", name=f"atf_{i}")
                state[("atf", i)] = atf
                for h in range(HEADS):
                    with nc.named_scope(f"attn_{i}_{h}"):
                        et = state[("et", i)][h]
                        rcb = rcp.tile([P, NH, 512], F32, tag="rcb",
                                       name=f"rcb_{i}_{h}")
                        for xh in range(NH):
                            sl = slice(xh * 512, (xh + 1) * 512)
                            at = ps_sml.tile([P, 512], F32, tag="mm",
                                             name="at")
                            sm = ps_sml.tile([P, 512], F32, tag="mm",
                                             name="sm")
                            for yp in range(0, YT, 2):
                                nc.tensor.matmul(
                                    at[:],
                                    vaug[:, yp:yp + 2, h, :],
                                    et[:, yp:yp + 2, sl],
                                    start=(yp == 0), stop=(yp == YT - 2),
                                    perf_mode=DR,
                                )
                                nc.tensor.matmul(
                                    sm[:],
                                    ones2[:, :, :],
                                    et[:, yp:yp + 2, sl],
                                    start=(yp == 0), stop=(yp == YT - 2),
                                    perf_mode=DR,
                                )
                            nc.vector.reciprocal_approx_fast(
                                rcb[:, xh, :], sm[:])
                            nc.vector.scalar_tensor_tensor(
                                atf[:, h, sl], at[:], 0.0, rcb[:, xh, :],
                                ALU.max, ALU.mult)

            def conv3(i, ots, tail):
                """tail='split'|'vector'"""
                atf = state[("atf", i)]
                xres_tiles = state[("xres", i)]
                with nc.named_scope(f"conv3_{i}_{ots[0]}"):
                    o_sb = None
                    for ot in ots:
                        if ot % 2 == 0:
                            o_sb = outp.tile([P, 2, HW], F16, tag="o")
                        for nh in range(NH):
                            sl = slice(nh * 512, (nh + 1) * 512)
                            ps = ps_sml.tile([P, 512], F32, tag="mm")
                            for kp in range(0, KT2, 2):
                                nc.tensor.matmul(
                                    ps[:],
                                    w3t[:, kp:kp + 2, ot * P:(ot + 1) * P],
                                    atf[:, kp:kp + 2, sl],
                                    start=(kp == 0), stop=False,
                                    perf_mode=DR,
                                )
                            nc.tensor.matmul(
                                ps[:], ident[:],
                                xres_tiles[ot // NCH][:, ot % NCH, sl],
                                start=False, stop=True)
                            if tail == "split" and nh == 0:
                                nc.scalar.activation(
                                    o_sb[:, ot % 2, sl], ps[:], AF.Relu,
                                    scale=1.0 / RES)
                            else:
                                nc.vector.tensor_scalar(
                                    o_sb[:, ot % 2, sl], ps[:], 1.0 / RES,
                                    0.0, ALU.mult, ALU.max)
                        if ot % 2 == 1 and i == 1 and ot >= 11:
                            for half in range(2):
                                [nc.gpsimd, nc.sync][(ot + half) % 2].dma_start(
                                    out_d.ap()[i, ot // 2][:, half, :],
                                    o_sb[:, half, :])
                        elif ot % 2 == 1:
                            [nc.gpsimd, nc.sync][(ot // 2) % 2].dma_start(
                                out_d.ap()[i, ot // 2], o_sb[:])

            # ---- schedule (image-interleaved) ----
            loadx(0, [nc.sync, nc.scalar], with_w1t=True, fine=True)
            load_weights()
            conv1(0, evict="vector")
            qk(0)
            logits_exp(0)
            vstage(0)
            xres_issue(0, [nc.gpsimd, nc.scalar])
            loadx(1, [nc.sync, nc.gpsimd])
            conv1(1, evict="vector")     # fills exp(0) window
            atsm(0)
            conv3(0, range(0, 6), tail="split")
            qk(1)
            logits_exp(1)
            conv3(0, range(6, 16), tail="vector")  # fills exp(1) window
            vstage(1)
            xres_issue(1, [nc.gpsimd, nc.scalar])
            atsm(1)
            conv3(1, range(OT3), tail="split")

    nc.compile()
    _BUILT["nc"] = nc
    return nc


def _prep_maps(x, conv1_w, gamma1, beta1, mean1, var1, qk_w, v_w, pos_h, pos_w,
               conv3_w, gamma3, beta3, mean3, var3):
    f16 = np.float16
    f32 = np.float32
    inv1 = (gamma1 / np.sqrt(var1 + EPS)).astype(f32)
    add1 = (beta1 - mean1 * inv1).astype(f32)
    inv3 = (gamma3 / np.sqrt(var3 + EPS)).astype(f32)
    add3 = (beta3 - mean3 * inv3).astype(f32)

    def q8(a, s):
        return np.ascontiguousarray(a * s).astype(NP_F8)

    NCH = 4
    NC1 = KT1 // NCH
    # [NC1, P, NCH, MID]: 4 k-tiles interleaved per partition row
    w1t = q8(conv1_w.T.reshape(NC1, NCH, P, MID).transpose(0, 2, 1, 3), SW1)
    qk_mod = np.concatenate(
        [qk_w[:HEADS * DQK] * SCALE, qk_w[HEADS * DQK:]], 0) * (inv1 / SW1)[None, :]
    qkwt = q8(qk_mod.T.reshape(KT2, P, 2 * MID).transpose(1, 0, 2), SW2)
    vwt = q8((v_w * (inv1 / SW1)[None, :]).T.reshape(KT2, P, MID).transpose(1, 0, 2), SW2)
    w3t = q8((conv3_w * inv3[:, None]).T.reshape(KT2, P, CIN).transpose(1, 0, 2), SW3)
    embt = np.ascontiguousarray(
        (pos_h[:, None, :] + pos_w[None, :, :]).reshape(HW, DQK).T * SW2
    ).astype(f32)
    # feat = relu(psum + SW1*(add1/inv1)) on device
    b1 = np.ascontiguousarray(
        (SW1 * add1 / inv1).reshape(OT1, P).T).astype(f32)

    xs = x.reshape(N_IMG, KT1, P, HW)
    xsc = xs.reshape(N_IMG, NC1, NCH, P, HW).transpose(0, 1, 3, 2, 4)
    x8_all = xsc.astype(NP_F8)
    xr_all = (xsc + add3.reshape(NC1, NCH, P).transpose(0, 2, 1)[None, :, :, :, None]
              ).astype(f16)

    in_maps = []
    for c in range(NCORES):
        sl = slice(c * IMGS_PER_CORE, (c + 1) * IMGS_PER_CORE)
        in_maps.append({
            "x8": np.ascontiguousarray(x8_all[sl]),
            "xr": np.ascontiguousarray(xr_all[sl]),
            "w1t": w1t, "qkwt": qkwt, "vwt": vwt, "w3t": w3t,
            "embt": embt, "b1": b1,
        })
    return in_maps


def _postprocess(out):
    # out: [N, OT3//2, P, 2, HW]; channel c = (pair*2 + half)*P + p
    out = out.transpose(0, 1, 3, 2, 4)
    return out.reshape(N_IMG, CIN, H, W).astype(np.float32)


def kernel(**inputs):
    nc = _build()
    inputs = {k: np.asarray(v) for k, v in inputs.items()}
    in_maps = _prep_maps(**inputs)
    res = run_bass_kernel_spmd(nc, in_maps, core_ids=list(range(NCORES)))
    out = np.concatenate([r["out"] for r in res.results], 0)
    return _postprocess(out)
